# revision 2
# baseline (speedup 1.0000x reference)
"""Fused LoRA-MLP (SwiGLU) expert kernel for TRN2, 8-core expert-parallel.

Problem (per full batch): x:(8192,2048) shared-weight expert MLP
    gu  = x @ W_gu.T + 0.25 * (x @ A_gu.T) @ B_gu.T        (.,8192)
    h   = gu[:, 4096:] * silu(gu[:, :4096])                 (.,4096)
    out = h @ W_d.T  + 0.25 * (h @ A_d.T)  @ B_d.T          (.,2048)

Sharding: expert/data parallel — core c owns tokens [1024c, 1024(c+1)),
weights replicated per core. No collectives.

All tensors are pre-transposed/pre-tiled on the host so the device kernel
needs zero on-chip transposes; activations flow feature-major
(xT -> guT -> hT -> outT). Weights and activations are bf16 (PSUM
accumulation stays fp32); output is written bf16 and upcast on host.
DMA is spread across both HWDGE queues (sync + scalar).
"""

import os
from contextlib import ExitStack

import numpy as np

import concourse.bass as bass
import concourse.bacc as bacc
import concourse.tile as tile
import concourse.mybir as mybir
from concourse.bass_utils import run_bass_kernel_spmd

F32 = mybir.dt.float32
BF16 = mybir.dt.bfloat16
AF = mybir.ActivationFunctionType

NCORES = 8
T = 1024          # tokens per core
H = 2048          # hidden
D = 4096          # expert dim
F = 2 * D         # gate+up features
R = 64            # lora rank
SCALING = 16 / 64

KT = H // 128     # 16 k-tiles (mm1 contraction)
FT = F // 128     # 64 f-tiles (mm1 outputs)
DT = D // 128     # 32 d-tiles (mm2 contraction)
JT = H // 128     # 16 j-tiles (mm2 outputs)
NB = 8            # mm2 d-blocks (4 d-tiles each)
TC = 512          # moving-dim chunk
NCH = T // TC     # 2 chunks
SLAB = KT * 128 + 128  # wgu slab cols: 16 k-tiles + packed bgu tile

_CACHE = {}


def _build(reps=1):
    nc = bacc.Bacc("TRN2", target_bir_lowering=False, debug=False,
                   num_devices=NCORES)

    xT = nc.dram_tensor("xT", [128, KT * T], BF16, kind="ExternalInput")
    wgu = nc.dram_tensor("wgu", [FT, 128, SLAB], BF16, kind="ExternalInput")
    agu = nc.dram_tensor("agu", [128, KT * R], BF16, kind="ExternalInput")
    wd = nc.dram_tensor("wd", [NB, JT, 128, 4 * 128], BF16, kind="ExternalInput")
    ad = nc.dram_tensor("ad", [128, DT * R], BF16, kind="ExternalInput")
    bd = nc.dram_tensor("bd", [JT, R, 128], BF16, kind="ExternalInput")
    outT = nc.dram_tensor("outT", [JT, 128, T], BF16, kind="ExternalOutput")

    with tile.TileContext(nc) as tc, ExitStack() as ctx:
        const = ctx.enter_context(tc.tile_pool(name="const", bufs=1))
        xpool = ctx.enter_context(tc.tile_pool(name="xpool", bufs=1))
        wgu_pool = ctx.enter_context(tc.tile_pool(name="wgup", bufs=4))
        wd_pool = ctx.enter_context(tc.tile_pool(name="wdp", bufs=3))
        bd_pool = ctx.enter_context(tc.tile_pool(name="bdp", bufs=2))
        ht_pool = ctx.enter_context(tc.tile_pool(name="htp", bufs=5))
        oacc_pool = ctx.enter_context(tc.tile_pool(name="oaccp", bufs=JT))
        obf_pool = ctx.enter_context(tc.tile_pool(name="obfp", bufs=2))
        sil_pool = ctx.enter_context(tc.tile_pool(name="silp", bufs=2))
        sm_pool = ctx.enter_context(tc.tile_pool(name="smp", bufs=1))
        ps_a = ctx.enter_context(tc.tile_pool(name="psa", bufs=4, space="PSUM"))
        ps_b = ctx.enter_context(tc.tile_pool(name="psb", bufs=2, space="PSUM"))
        ps_c = ctx.enter_context(tc.tile_pool(name="psc", bufs=2, space="PSUM"))

        for rep in range(reps):
            # resident small weights
            abuf = const.tile([128, KT * R], BF16)
            nc.sync.dma_start(out=abuf[:], in_=agu[:, :])
            adbuf = const.tile([128, DT * R], BF16)
            nc.sync.dma_start(out=adbuf[:], in_=ad[:, :])

            # xT resident [128, kt*1024]
            xbuf = xpool.tile([128, KT * T], BF16)
            nc.scalar.dma_start(out=xbuf[:], in_=xT[:, :])

            def xsl(k, c):
                return xbuf[:, k * T + c * TC: k * T + (c + 1) * TC]

            # ---- lora-1: xaT [64, T] = A_gu @ x.T
            xa_sb = sm_pool.tile([R, T], BF16, tag="xa")
            for c in range(NCH):
                pxa = ps_b.tile([R, TC], F32, tag="psb")
                for k in range(KT):
                    nc.tensor.matmul(
                        pxa[:], abuf[:, k * R:(k + 1) * R], xsl(k, c),
                        start=(k == 0), stop=(k == KT - 1))
                nc.vector.tensor_copy(xa_sb[:, c * TC:(c + 1) * TC], pxa[:])

            # persistent xa2 accumulators (one bank per chunk)
            pxa2 = [ps_c.tile([R, TC], F32, tag="psc", name=f"pxa2_{rep}_{c}")
                    for c in range(NCH)]

            ht_tiles = [None] * DT
            oacc = [None] * JT

            def emit_xa2_mm(i):
                for c in range(NCH):
                    nc.tensor.matmul(
                        pxa2[c][:], adbuf[:, i * R:(i + 1) * R],
                        ht_tiles[i][:, c * TC:(c + 1) * TC],
                        start=(i == 0), stop=(i == DT - 1))

            def mm2_block(b, fuse_tail=False):
                for j in range(JT):
                    wdt = wd_pool.tile([128, 4 * 128], BF16, tag="wd")
                    (nc.sync if j % 2 else nc.scalar).dma_start(
                        out=wdt[:], in_=wd[b, j])
                    if fuse_tail:
                        bdt = bd_pool.tile([R, 128], BF16, tag="bd")
                        nc.sync.dma_start(out=bdt[:], in_=bd[j])
                        obf = obf_pool.tile([128, T], BF16, tag="obf")
                    for c in range(NCH):
                        ps = ps_b.tile([128, TC], F32, tag="psb")
                        for dt_ in range(4):
                            d = b * 4 + dt_
                            nc.tensor.matmul(
                                ps[:], wdt[:, dt_ * 128:(dt_ + 1) * 128],
                                ht_tiles[d][:, c * TC:(c + 1) * TC],
                                start=(dt_ == 0), stop=(dt_ == 3 and not fuse_tail))
                        if fuse_tail:
                            # lora-2 tail folded into the last accumulation
                            nc.tensor.matmul(
                                ps[:], bdt[:],
                                xa2_sb[:, c * TC:(c + 1) * TC],
                                start=False, stop=True)
                        dst = oacc[j][:, c * TC:(c + 1) * TC]
                        if b == 0:
                            nc.vector.tensor_copy(dst, ps[:])
                        elif fuse_tail:
                            nc.vector.tensor_add(
                                obf[:, c * TC:(c + 1) * TC], dst, ps[:])
                        else:
                            nc.vector.tensor_add(dst, dst, ps[:])
                    if fuse_tail:
                        nc.sync.dma_start(out=outT[j], in_=obf[:])

            # ---- main mm1 loop over f-pairs (gate m=i, up m=i+32)
            for i in range(DT):
                # consumers lagged one pair so PE never waits on fresh DVE output
                if i > 0:
                    emit_xa2_mm(i - 1)
                if i % 4 == 0 and i > 0:
                    mm2_block(i // 4 - 1)

                slabs = {}
                for m in (i, i + DT):
                    s = wgu_pool.tile([128, SLAB], BF16, tag="wgu")
                    (nc.sync if m % 2 else nc.scalar).dma_start(
                        out=s[:], in_=wgu[m])
                    slabs[m] = s

                ht_i = ht_pool.tile([128, T], BF16, tag="ht")
                ht_tiles[i] = ht_i
                for c in range(NCH):
                    pg = ps_a.tile([128, TC], F32, tag="psa")
                    pu = ps_a.tile([128, TC], F32, tag="psa")
                    for ps, m in ((pg, i), (pu, i + DT)):
                        s = slabs[m]
                        for k in range(KT):
                            nc.tensor.matmul(
                                ps[:], s[:, k * 128:(k + 1) * 128],
                                xsl(k, c), start=(k == 0), stop=False)
                        nc.tensor.matmul(
                            ps[:], s[0:R, KT * 128:KT * 128 + 128],
                            xa_sb[:, c * TC:(c + 1) * TC],
                            start=False, stop=True)
                    sil = sil_pool.tile([128, TC], F32, tag="sil")
                    nc.scalar.activation(sil[:], pg[:], AF.Silu)
                    nc.vector.tensor_mul(ht_i[:, c * TC:(c + 1) * TC], pu[:], sil[:])

                if i == 0:
                    for j in range(JT):
                        oacc[j] = oacc_pool.tile([128, T], F32, tag="oacc",
                                                 name=f"oacc_{rep}_{j}")

            emit_xa2_mm(DT - 1)

            # ---- lora-2: xa2 to SBUF, then final mm2 block fused with
            # the B_d tail and the bf16 output store
            xa2_sb = sm_pool.tile([R, T], BF16, tag="xa2")
            for c in range(NCH):
                nc.vector.tensor_copy(xa2_sb[:, c * TC:(c + 1) * TC], pxa2[c][:])
            mm2_block(NB - 1, fuse_tail=True)

    nc.compile()
    return nc


def _prep_shared(W_gu, A_gu, B_gu, W_d, A_d, B_d):
    bf = mybir.dt.np(BF16)
    # wgu slab [m, p, SLAB]: cols 0..2048 = W_gu.T tiles, cols 2048..2176
    # rows 0..63 = scaled B_gu.T tile for the same f-tile m
    wgu_t = np.zeros((FT, 128, SLAB), np.float32)
    wgu_t[:, :, :KT * 128] = W_gu.reshape(FT, 128, KT, 128).transpose(
        0, 3, 2, 1).reshape(FT, 128, KT * 128)
    wgu_t[:, :R, KT * 128:KT * 128 + 128] = (
        (B_gu * SCALING).reshape(FT, 128, R).transpose(0, 2, 1))
    agu_t = np.ascontiguousarray(
        A_gu.T.reshape(KT, 128, R).transpose(1, 0, 2)).reshape(128, KT * R)
    wd_t = np.ascontiguousarray(
        W_d.reshape(JT, 128, NB, 4, 128).transpose(2, 0, 4, 3, 1)
    ).reshape(NB, JT, 128, 4 * 128)
    ad_t = np.ascontiguousarray(
        A_d.T.reshape(DT, 128, R).transpose(1, 0, 2)).reshape(128, DT * R)
    bd_t = np.ascontiguousarray(
        (B_d * SCALING).reshape(JT, 128, R).transpose(0, 2, 1))
    return dict(wgu=wgu_t.astype(bf), agu=agu_t.astype(bf),
                wd=wd_t.astype(bf), ad=ad_t.astype(bf), bd=bd_t.astype(bf))


def kernel(hidden_states, W_gu, A_gu, B_gu, W_d, A_d, B_d):
    bf = mybir.dt.np(BF16)
    hidden_states = np.asarray(hidden_states, dtype=np.float32)
    shared = _prep_shared(*(np.asarray(a, dtype=np.float32)
                            for a in (W_gu, A_gu, B_gu, W_d, A_d, B_d)))

    # per-core xT pre-tiled as [p, k, t] flattened to [128, KT*T]
    xt = np.ascontiguousarray(
        hidden_states.reshape(NCORES, T, KT, 128).transpose(0, 3, 2, 1)
    ).reshape(NCORES, 128, KT * T).astype(bf)

    if "nc" not in _CACHE:
        _CACHE["nc"] = _build()
    nc = _CACHE["nc"]

    in_maps = [dict(shared, xT=xt[c]) for c in range(NCORES)]
    trace = os.environ.get("KERNEL_TRACE", "0") == "1"
    res = run_bass_kernel_spmd(nc, in_maps, list(range(NCORES)), trace=trace)
    _CACHE["last_result"] = res

    out = np.empty((NCORES, T, H), np.float32)
    for c in range(NCORES):
        o = res.results[c]["outT"].astype(np.float32).reshape(JT, 128, T)
        out[c] = o.transpose(2, 0, 1).reshape(T, H)
    return out.reshape(NCORES * T, H)


# revision 10
# speedup vs baseline: 530.3421x; 530.3421x over previous
"""Fused LoRA-MLP (SwiGLU) expert kernel for TRN2, 8-core expert-parallel.

Problem (per full batch): x:(8192,2048) shared-weight expert MLP
    gu  = x @ W_gu.T + 0.25 * (x @ A_gu.T) @ B_gu.T        (.,8192)
    h   = gu[:, 4096:] * silu(gu[:, :4096])                 (.,4096)
    out = h @ W_d.T  + 0.25 * (h @ A_d.T)  @ B_d.T          (.,2048)

Sharding: expert/data parallel — core c owns tokens [1024c, 1024(c+1)),
weights replicated per core. No collectives.

All tensors are pre-transposed/pre-tiled on the host so the device kernel
needs zero on-chip transposes; activations flow feature-major
(xT -> guT -> hT -> outT). Weights and activations are bf16 (PSUM
accumulation stays fp32); output is written bf16 and upcast on host.
DMA is spread across both HWDGE queues (sync + scalar).
"""

import os
from contextlib import ExitStack

import numpy as np

import concourse.bass as bass
import concourse.bacc as bacc
import concourse.tile as tile
import concourse.mybir as mybir
from concourse.bass_utils import run_bass_kernel_spmd

F32 = mybir.dt.float32
BF16 = mybir.dt.bfloat16
AF = mybir.ActivationFunctionType

NCORES = 8
T = 1024          # tokens per core
H = 2048          # hidden
D = 4096          # expert dim
F = 2 * D         # gate+up features
R = 64            # lora rank
SCALING = 16 / 64

KT = H // 128     # 16 k-tiles (mm1 contraction)
FT = F // 128     # 64 f-tiles (mm1 outputs)
DT = D // 128     # 32 d-tiles (mm2 contraction)
JT = H // 128     # 16 j-tiles (mm2 outputs)
NB = 8            # mm2 d-blocks (4 d-tiles each)
TC = 512          # moving-dim chunk
NCH = T // TC     # 2 chunks
SLAB = KT * 128 + 128  # wgu slab cols: 16 k-tiles + packed bgu tile

_CACHE = {}


def _build(reps=1, loop_n=None):
    nc = bacc.Bacc("TRN2", target_bir_lowering=False, debug=False,
                   num_devices=NCORES)

    xT = nc.dram_tensor("xT", [128, KT * T], BF16, kind="ExternalInput")
    wgu = nc.dram_tensor("wgu", [FT, 128, SLAB], BF16, kind="ExternalInput")
    agu = nc.dram_tensor("agu", [128, KT * R], BF16, kind="ExternalInput")
    wd = nc.dram_tensor("wd", [NB, JT, 128, 4 * 128], BF16, kind="ExternalInput")
    ad = nc.dram_tensor("ad", [128, DT * R], BF16, kind="ExternalInput")
    bd = nc.dram_tensor("bd", [JT, R, 128], BF16, kind="ExternalInput")
    outT = nc.dram_tensor("outT", [JT, 128, T], BF16, kind="ExternalOutput")

    with tile.TileContext(nc) as tc, ExitStack() as ctx:
        const = ctx.enter_context(tc.tile_pool(name="const", bufs=1))
        xpool = ctx.enter_context(tc.tile_pool(name="xpool", bufs=1))
        wgu_pool = ctx.enter_context(tc.tile_pool(name="wgup", bufs=4))
        wd_pool = ctx.enter_context(tc.tile_pool(name="wdp", bufs=4))
        bd_pool = ctx.enter_context(tc.tile_pool(name="bdp", bufs=2))
        ht_pool = ctx.enter_context(tc.tile_pool(name="htp", bufs=8))
        oacc_pool = ctx.enter_context(tc.tile_pool(name="oaccp", bufs=JT))
        obf_pool = ctx.enter_context(tc.tile_pool(name="obfp", bufs=2))
        sil_pool = ctx.enter_context(tc.tile_pool(name="silp", bufs=2))
        sm_pool = ctx.enter_context(tc.tile_pool(name="smp", bufs=1))
        ps_a = ctx.enter_context(tc.tile_pool(name="psa", bufs=4, space="PSUM"))
        ps_b = ctx.enter_context(tc.tile_pool(name="psb", bufs=2, space="PSUM"))
        ps_c = ctx.enter_context(tc.tile_pool(name="psc", bufs=2, space="PSUM"))

        def emit_rep(rep):
            # resident small weights
            abuf = const.tile([128, KT * R], BF16)
            nc.sync.dma_start(out=abuf[:], in_=agu[:, :])
            adbuf = const.tile([128, DT * R], BF16)
            nc.sync.dma_start(out=adbuf[:], in_=ad[:, :])

            # xT resident [128, kt*1024]
            xbuf = xpool.tile([128, KT * T], BF16)
            nc.scalar.dma_start(out=xbuf[:], in_=xT[:, :])

            def xsl(k, c):
                return xbuf[:, k * T + c * TC: k * T + (c + 1) * TC]

            # ---- lora-1: xaT [64, T] = A_gu @ x.T
            xa_sb = sm_pool.tile([R, T], BF16, tag="xa")
            for c in range(NCH):
                pxa = ps_b.tile([R, TC], F32, tag="psb")
                for k in range(KT):
                    nc.tensor.matmul(
                        pxa[:], abuf[:, k * R:(k + 1) * R], xsl(k, c),
                        start=(k == 0), stop=(k == KT - 1))
                nc.vector.tensor_copy(xa_sb[:, c * TC:(c + 1) * TC], pxa[:])

            # persistent xa2 accumulators (one bank per chunk)
            pxa2 = [ps_c.tile([R, TC], F32, tag="psc", name=f"pxa2_{rep}_{c}")
                    for c in range(NCH)]

            ht_tiles = [None] * DT
            oacc = [None] * JT

            def emit_xa2_mm(i):
                for c in range(NCH):
                    nc.tensor.matmul(
                        pxa2[c][:], adbuf[:, i * R:(i + 1) * R],
                        ht_tiles[i][:, c * TC:(c + 1) * TC],
                        start=(i == 0), stop=(i == DT - 1))

            def mm2_block(b, fuse_tail=False):
                for j in range(JT):
                    wdt = wd_pool.tile([128, 4 * 128], BF16, tag="wd")
                    (nc.sync if j % 2 else nc.scalar).dma_start(
                        out=wdt[:], in_=wd[b, j])
                    if fuse_tail:
                        bdt = bd_pool.tile([R, 128], BF16, tag="bd")
                        nc.sync.dma_start(out=bdt[:], in_=bd[j])
                        obf = obf_pool.tile([128, T], BF16, tag="obf")
                    # both chunks under one weight load per d-tile
                    psl = [ps_b.tile([128, TC], F32, tag="psb",
                                     name=f"ps2_{b}_{j}_{c}")
                           for c in range(NCH)]
                    for dt_ in range(4):
                        d = b * 4 + dt_
                        for c in range(NCH):
                            nc.tensor.matmul(
                                psl[c][:], wdt[:, dt_ * 128:(dt_ + 1) * 128],
                                ht_tiles[d][:, c * TC:(c + 1) * TC],
                                start=(dt_ == 0), stop=(dt_ == 3 and not fuse_tail))
                    if fuse_tail:
                        # lora-2 tail folded into the last accumulation
                        for c in range(NCH):
                            nc.tensor.matmul(
                                psl[c][:], bdt[:],
                                xa2_sb[:, c * TC:(c + 1) * TC],
                                start=False, stop=True)
                    for c in range(NCH):
                        dst = oacc[j][:, c * TC:(c + 1) * TC]
                        if b == 0:
                            nc.vector.tensor_copy(dst, psl[c][:])
                        elif fuse_tail:
                            nc.vector.tensor_add(
                                obf[:, c * TC:(c + 1) * TC], dst, psl[c][:])
                        else:
                            nc.vector.tensor_add(dst, dst, psl[c][:])
                    if fuse_tail:
                        nc.sync.dma_start(out=outT[j], in_=obf[:])

            # ---- main mm1 loop over f-pairs (gate m=i, up m=i+32)
            for i in range(DT):
                # consumers lagged one pair so PE never waits on fresh DVE output
                if i > 0:
                    emit_xa2_mm(i - 1)
                if i % 4 == 0 and i > 0:
                    mm2_block(i // 4 - 1)

                slabs = {}
                for m in (i, i + DT):
                    s = wgu_pool.tile([128, SLAB], BF16, tag="wgu")
                    (nc.sync if m % 2 else nc.scalar).dma_start(
                        out=s[:], in_=wgu[m])
                    slabs[m] = s

                ht_i = ht_pool.tile([128, T], BF16, tag="ht")
                ht_tiles[i] = ht_i
                # both chunks under one weight load per k-tile
                pp = {}
                for m in (i, i + DT):
                    s = slabs[m]
                    psl = [ps_a.tile([128, TC], F32, tag="psa",
                                     name=f"ps1_{i}_{m}_{c}")
                           for c in range(NCH)]
                    for k in range(KT):
                        for c in range(NCH):
                            nc.tensor.matmul(
                                psl[c][:], s[:, k * 128:(k + 1) * 128],
                                xsl(k, c), start=(k == 0), stop=False)
                    for c in range(NCH):
                        nc.tensor.matmul(
                            psl[c][:], s[0:R, KT * 128:KT * 128 + 128],
                            xa_sb[:, c * TC:(c + 1) * TC],
                            start=False, stop=True)
                    pp[m] = psl
                for c in range(NCH):
                    sil = sil_pool.tile([128, TC], F32, tag="sil")
                    nc.scalar.activation(sil[:], pp[i][c][:], AF.Silu)
                    nc.vector.tensor_mul(ht_i[:, c * TC:(c + 1) * TC],
                                         pp[i + DT][c][:], sil[:])

                if i == 0:
                    for j in range(JT):
                        oacc[j] = oacc_pool.tile([128, T], F32, tag="oacc",
                                                 name=f"oacc_{rep}_{j}")

            emit_xa2_mm(DT - 1)

            # ---- lora-2: xa2 to SBUF, then final mm2 block fused with
            # the B_d tail and the bf16 output store
            xa2_sb = sm_pool.tile([R, T], BF16, tag="xa2")
            for c in range(NCH):
                nc.vector.tensor_copy(xa2_sb[:, c * TC:(c + 1) * TC], pxa2[c][:])
            mm2_block(NB - 1, fuse_tail=True)

        if loop_n is not None:
            with tc.For_i(0, loop_n):
                emit_rep(0)
        else:
            for rep in range(reps):
                emit_rep(rep)

    nc.compile()
    return nc


def _prep_shared(W_gu, A_gu, B_gu, W_d, A_d, B_d):
    bf = mybir.dt.np(BF16)
    # wgu slab [m, p, SLAB]: cols 0..2048 = W_gu.T tiles, cols 2048..2176
    # rows 0..63 = scaled B_gu.T tile for the same f-tile m
    wgu_t = np.zeros((FT, 128, SLAB), np.float32)
    wgu_t[:, :, :KT * 128] = W_gu.reshape(FT, 128, KT, 128).transpose(
        0, 3, 2, 1).reshape(FT, 128, KT * 128)
    wgu_t[:, :R, KT * 128:KT * 128 + 128] = (
        (B_gu * SCALING).reshape(FT, 128, R).transpose(0, 2, 1))
    agu_t = np.ascontiguousarray(
        A_gu.T.reshape(KT, 128, R).transpose(1, 0, 2)).reshape(128, KT * R)
    wd_t = np.ascontiguousarray(
        W_d.reshape(JT, 128, NB, 4, 128).transpose(2, 0, 4, 3, 1)
    ).reshape(NB, JT, 128, 4 * 128)
    ad_t = np.ascontiguousarray(
        A_d.T.reshape(DT, 128, R).transpose(1, 0, 2)).reshape(128, DT * R)
    bd_t = np.ascontiguousarray(
        (B_d * SCALING).reshape(JT, 128, R).transpose(0, 2, 1))
    return dict(wgu=wgu_t.astype(bf), agu=agu_t.astype(bf),
                wd=wd_t.astype(bf), ad=ad_t.astype(bf), bd=bd_t.astype(bf))


def kernel(hidden_states, W_gu, A_gu, B_gu, W_d, A_d, B_d):
    bf = mybir.dt.np(BF16)
    hidden_states = np.asarray(hidden_states, dtype=np.float32)
    shared = _prep_shared(*(np.asarray(a, dtype=np.float32)
                            for a in (W_gu, A_gu, B_gu, W_d, A_d, B_d)))

    # per-core xT pre-tiled as [p, k, t] flattened to [128, KT*T]
    xt = np.ascontiguousarray(
        hidden_states.reshape(NCORES, T, KT, 128).transpose(0, 3, 2, 1)
    ).reshape(NCORES, 128, KT * T).astype(bf)

    if "nc" not in _CACHE:
        _CACHE["nc"] = _build()
    nc = _CACHE["nc"]

    in_maps = [dict(shared, xT=xt[c]) for c in range(NCORES)]
    trace = os.environ.get("KERNEL_TRACE", "0") == "1"
    res = run_bass_kernel_spmd(nc, in_maps, list(range(NCORES)), trace=trace)
    _CACHE["last_result"] = res

    out = np.empty((NCORES, T, H), np.float32)
    for c in range(NCORES):
        o = res.results[c]["outT"].astype(np.float32).reshape(JT, 128, T)
        out[c] = o.transpose(2, 0, 1).reshape(T, H)
    return out.reshape(NCORES * T, H)
